# revision 37
# baseline (speedup 1.0000x reference)
"""Trainium2 Bass kernel for the GCN message-passing model (8 NeuronCores).

Strategy (graph/data parallel per the sharding hint):
- Pad nodes 50000 -> 50176 = 8*49*128; permute node ids so every 128-node
  dst tile has a near-equal number of incoming edges (balanced one-hot
  matmul chunks), shard nodes 6272/core.
- Dense layers run feature-major (features on partitions) so bias+leaky_relu
  fuse into one ScalarEngine activation per block.
- Each GCN conv: per-core hW = (x @ W) * dinv[src] computed node-major,
  cast bf16, AllGather -> full [50176, F] gather table in every core's HBM.
  Self-loops are folded in as ordinary edges (out = lrelu(dinv*(agg) + b)
  with agg including the dinv*hW self term).
- Per dst tile: dma_gather the incoming edges' rows (int16 idx, split at
  row 32768), build one-hot S from uploaded dst-locals, segment-sum via
  PE matmuls accumulating in PSUM, epilogue on DVE+ACT.
- Convs are fused per round: round1 = {g1, f1, ns1} (384 gathered cols),
  round2 = {g2, f2} (256 cols) — one gather/S/matmul pass per round.
"""
import os
import os
import sys
import types

import numpy as np

N_REAL = 50000
NCORES = 8
NT = 49                      # dst tiles per core
SH = NT * 128                # 6272 nodes per core
NPAD = NCORES * SH           # 50176
SPLIT = 32768                # int16 gather index limit
F1, F2 = 384, 256            # gathered cols round1 / round2
S_MODE = "bcast"             # "bcast" | "chunk" one-hot build
NQUEUES = int(os.environ.get("K_NQUEUES", "2"))
COPY_AG = os.environ.get("K_COPY_AG", "0") == "1"
AGSPLIT = int(os.environ.get("K_AGSPLIT", "2"))
NQUEUES = int(os.environ.get("K_NQUEUES", "2"))
COPY_AG = os.environ.get("K_COPY_AG", "0") == "1"
AGSPLIT = int(os.environ.get("K_AGSPLIT", "2"))


def _setup_paths():
    if '/opt/trn_rl_repo' not in sys.path:
        sys.path.insert(0, '/opt/trn_rl_repo')
    if '/root/.axon_site' not in sys.path:
        sys.path.insert(0, '/root/.axon_site')
    if 'antenv.axon_hooks' not in sys.modules:
        box = [None]
        m = types.ModuleType('antenv.axon_hooks')
        m.set_axon_ntff_profile_hook = lambda h: box.__setitem__(0, h)
        m.get_axon_ntff_profile_hook = lambda: box[0]
        sys.modules['antenv.axon_hooks'] = m
        try:
            from trn_agent_boot.trn_boot import _ntff_profile_via_ctypes
            m.set_axon_ntff_profile_hook(
                _ntff_profile_via_ctypes('/opt/axon/libaxon_pjrt.so'))
        except Exception:
            pass


# ----------------------------------------------------------------------------
# Host-side graph preprocessing
# ----------------------------------------------------------------------------

def _balance_nodes(w_hi, w_lo):
    """Assign NPAD nodes to NT*NCORES tiles of 128, balancing the per-tile
    high-src edge count (tight: cap 6*128) primarily and total secondarily.
    Returns new2old [NPAD] (node order: tile t slots t*128..t*128+127)."""
    ntiles = NPAD // 128
    order = np.argsort(-(w_hi * 64 + w_lo), kind="stable")
    import heapq
    heap = [(0.0, t) for t in range(ntiles)]
    heapq.heapify(heap)
    counts = np.zeros(ntiles, np.int64)
    members = [[] for _ in range(ntiles)]
    for node in order:
        while True:
            s, t = heapq.heappop(heap)
            if counts[t] < 128:
                break
        members[t].append(node)
        counts[t] += 1
        if counts[t] < 128:
            heapq.heappush(heap, (s + float(w_hi[node]) + float(w_lo[node]) / 64.0, t))
    new2old = np.concatenate([np.array(m, np.int64) for m in members])
    return new2old


def _prep_graph(edge_index):
    src = np.asarray(edge_index[0], np.int64)
    dst = np.asarray(edge_index[1], np.int64)
    deg = np.bincount(dst, minlength=NPAD).astype(np.float32) + 1.0
    deg[N_REAL:] = 1.0
    dinv = (deg ** -0.5).astype(np.float32)

    # augmented edge list: real edges + one self loop per real node
    srcA = np.concatenate([src, np.arange(N_REAL)])
    dstA = np.concatenate([dst, np.arange(N_REAL)])

    # per-dst counts split by whether the src new-id will be >= SPLIT is
    # circular; approximate with old src ids (the permutation preserves the
    # low/high population ratio closely).
    hi_mask = srcA >= SPLIT
    w_hi = np.bincount(dstA[hi_mask], minlength=NPAD)
    w_lo = np.bincount(dstA[~hi_mask], minlength=NPAD)
    new2old = _balance_nodes(w_hi, w_lo)
    old2new = np.empty(NPAD, np.int64)
    old2new[new2old] = np.arange(NPAD)

    s_new = old2new[srcA]
    if AGSPLIT > 1:
        rows = SH // AGSPLIT
        r = s_new // SH
        i = s_new % SH
        s_new = (i // rows) * (NPAD // AGSPLIT) + r * rows + (i % rows)
    d_new = old2new[dstA]
    tile = d_new >> 7
    dloc = d_new & 127

    # group edges by (tile, low/high src)
    ntiles = NPAD // 128
    okey = tile * 2 + (s_new >= SPLIT)
    eorder = np.argsort(okey, kind="stable")
    s_new, dloc, okey = s_new[eorder], dloc[eorder], okey[eorder]
    n_low = np.bincount(tile[eorder] * 2 + (s_new >= SPLIT),
                        minlength=ntiles * 2).reshape(ntiles, 2)
    starts = np.zeros(ntiles * 2 + 1, np.int64)
    np.cumsum(n_low.reshape(-1), out=starts[1:])
    n_lo, n_hi = n_low[:, 0], n_low[:, 1]

    # per tile-slot chunk structure, maxed over cores (SPMD: one graph)
    n_lo_s = n_lo.reshape(NCORES, NT)
    n_hi_s = n_hi.reshape(NCORES, NT)
    CL = np.maximum(1, (n_lo_s.max(axis=0) + 127) // 128).astype(np.int64)
    CH = np.maximum(1, (n_hi_s.max(axis=0) + 127) // 128).astype(np.int64)
    CT = CL + CH                                     # chunks per tile slot
    totc = int(CT.sum())                             # chunk columns per core

    # per-core packed idx (int16, wrapped 16 + replicated x8) and dstloc (f32)
    IDX = np.zeros((NCORES, 128, totc * 8), np.int16)
    import ml_dtypes
    DST = np.full((NCORES, 128, totc), 255.0, ml_dtypes.bfloat16)
    ccol = np.zeros(NT + 1, np.int64)
    np.cumsum(CT, out=ccol[1:])
    for c in range(NCORES):
        for t in range(NT):
            g = c * NT + t
            base = ccol[t]
            for half, (cnt, coff, clen) in enumerate(
                    [(int(n_lo[g]), 0, int(CL[t])),
                     (int(n_hi[g]), int(CL[t]), int(CH[t]))]):
                e0 = starts[g * 2 + half]
                sl = s_new[e0:e0 + cnt] - (SPLIT if half else 0)
                dl = dloc[e0:e0 + cnt]
                nslot = clen * 128
                ii = np.arange(cnt)
                wrapped = np.zeros((16, clen * 8), np.int16)
                wrapped[ii % 16, ii // 16] = sl.astype(np.int16)
                IDX[c, :, (base + coff) * 8:(base + coff + clen) * 8] = \
                    np.tile(wrapped, (8, 1))
                dcol = np.full((128, clen), 255.0, ml_dtypes.bfloat16)
                dcol[ii % 128, ii // 128] = dl.astype(np.float32).astype(ml_dtypes.bfloat16)
                DST[c, :, base + coff:base + coff + clen] = dcol

    dinv_new = dinv[new2old]
    DINV = dinv_new.reshape(NCORES, NT, 128).transpose(0, 2, 1).copy()  # [C,128,NT]
    return dict(new2old=new2old, old2new=old2new, dinv_new=dinv_new,
                CL=CL, CH=CH, CT=CT, ccol=ccol, totc=totc,
                IDX=IDX, DST=DST, DINV=DINV)


# ----------------------------------------------------------------------------
# Bass graph
# ----------------------------------------------------------------------------

def _build(g):
    import concourse.bass as bass
    import concourse.bacc as bacc
    import concourse.tile as tile
    import concourse.mybir as mybir

    f32 = mybir.dt.float32
    bf16 = mybir.dt.bfloat16
    i16 = mybir.dt.int16
    AF = mybir.ActivationFunctionType
    OP = mybir.AluOpType
    AP = bass.AP
    ts = bass.ts

    CL, CH, CT, ccol, totc = g["CL"], g["CH"], g["CT"], g["ccol"], g["totc"]
    BLK = max(d for d in range(1, 513) if SH % d == 0)
    NBLK = SH // BLK

    nc = bacc.Bacc("TRN2", target_bir_lowering=False, debug=False,
                   num_devices=NCORES, num_swdge_queues=NQUEUES)

    def inp(name, shape, dt=f32):
        return nc.dram_tensor(name, shape, dt, kind="ExternalInput")

    dx_t = inp("dx_fm", [64, SH])
    cx1_t = inp("cx1_fm", [13, SH])
    cx2_t = inp("cx2_fm", [13, SH])
    cd_t = inp("cd_fm", [16, SH])
    idx_t = inp("idx", [128, totc * 8], i16)
    dstloc_t = inp("dstloc", [128, totc], bf16)
    dinv_t = inp("dinv_nm", [128, NT])
    iota_t = inp("iota", [128, 128], bf16)
    ident_t = inp("ident", [128, 128])
    biasb1_t = inp("biasb1", [128, F1])
    biasb2_t = inp("biasb2", [128, F2])
    sel_t = inp("sel", [1, 4])
    wl2bf_t = inp("Wl2bf", [64, 1], bf16)
    wl4bf_t = inp("Wl4bf", [64, 1], bf16)
    wr2bf_t = inp("Wr2bf", [2, 1], bf16)
    selbf_t = inp("selbf", [1, 4], bf16)
    W = {}
    for nm, sh in [("Wd", [64, 128]), ("Wc1", [13, 128]), ("Wc2", [13, 128]),
                   ("Wg0", [64, 128]), ("Wf0", [64, 128]), ("Wns0", [16, 64]),
                   ("Wg1", [128, 128]), ("Wf1", [128, 128]), ("Wns1", [64, 128]),
                   ("Wg2", [128, 128]), ("Wf2", [128, 128]),
                   ("Wfus", [512, 128]), ("Wl1", [128, 64]), ("Wl2", [64, 1]),
                   ("Wl3", [128, 64]), ("Wl4", [64, 1]), ("Wr1", [2, 2]),
                   ("Wr2", [2, 1]),
                   ("bd", [128, 1]), ("bg0", [128, 1]), ("bf0", [128, 1]),
                   ("bns0", [64, 1]), ("bfus", [128, 1]), ("bl1", [64, 1]),
                   ("bl2", [1, 1]), ("bl3", [64, 1]), ("bl4", [1, 1]),
                   ("br1", [2, 1]), ("br2", [1, 1]),
                   ("bc1", [128, 1]), ("bc2", [128, 1])]:
        W[nm] = inp(nm, sh)

    out_t = nc.dram_tensor("out", [259, SH], f32, kind="ExternalOutput")

    # internal DRAM
    hw1_loc = nc.dram_tensor("hw1_loc", [SH, F1], bf16)
    hw1_all = nc.dram_tensor("hw1_all", [NPAD, F1], bf16, addr_space="Shared")
    hw2_loc = nc.dram_tensor("hw2_loc", [SH, F2], bf16)
    hw2_all = nc.dram_tensor("hw2_all", [NPAD, F2], bf16, addr_space="Shared")

    with tile.TileContext(nc) as tc:
      with (
          tc.tile_pool(name="persist", bufs=1) as pp,
          tc.tile_pool(name="wpool", bufs=1) as wp,
      ):
        idx = pp.tile([128, totc * 8], i16)
        dstloc = pp.tile([128, totc], bf16)
        dinv = pp.tile([128, NT], f32)
        iota = pp.tile([128, 128], bf16)
        ident = pp.tile([128, 128], f32)
        biasb1 = pp.tile([128, F1], f32)
        biasb2 = pp.tile([128, F2], f32)
        sel = pp.tile([1, 4], f32)
        wl2bf = pp.tile([64, 1], bf16)
        wl4bf = pp.tile([64, 1], bf16)
        wr2bf = pp.tile([2, 1], bf16)
        selbf = pp.tile([1, 4], bf16)
        nc.sync.dma_start(selbf[:], selbf_t[:])
        nc.sync.dma_start(sel[:], sel_t[:])
        nc.sync.dma_start(wl2bf[:], wl2bf_t[:])
        nc.sync.dma_start(wl4bf[:], wl4bf_t[:])
        nc.sync.dma_start(wr2bf[:], wr2bf_t[:])
        nc.sync.dma_start(idx[:], idx_t[:])
        nc.sync.dma_start(dstloc[:], dstloc_t[:])
        nc.sync.dma_start(dinv[:], dinv_t[:])
        nc.sync.dma_start(iota[:], iota_t[:])
        nc.sync.dma_start(ident[:], ident_t[:])
        nc.sync.dma_start(biasb1[:], biasb1_t[:])
        nc.sync.dma_start(biasb2[:], biasb2_t[:])
        Wsb = {}
        for nm, t in W.items():
            if nm == "Wfus":
                continue
            sh = t.shape
            Wsb[nm] = wp.tile(list(sh), f32, name=f"W_{nm}")
            nc.sync.dma_start(Wsb[nm][:], t[:])
        wfus = wp.tile([128, 4, 128], f32)
        for b in range(4):
            nc.sync.dma_start(wfus[:, b, :], W["Wfus"][b * 128:(b + 1) * 128, :])

        # ---------------- Phase A: x_g0, x_f0, x_ns0 (feature-major) --------
        with (
            tc.tile_pool(name="keepA", bufs=1) as ka,
            tc.tile_pool(name="streamA", bufs=3) as sa,
            tc.tile_pool(name="psumA", bufs=2, space="PSUM") as pa,
        ):
            x_g0 = ka.tile([128, SH], f32)
            x_f0 = ka.tile([128, SH], f32)
            x_ns0 = ka.tile([64, SH], f32)
            dx_sb = sa.tile([64, SH], f32, tag="dx_sb", bufs=1)
            cd_sb = sa.tile([16, SH], f32, tag="cd_sb", bufs=1)
            nc.sync.dma_start(dx_sb[:], dx_t[:, :])
            nc.sync.dma_start(cd_sb[:], cd_t[:, :])
            # interleave hW1 emission so split-AG1 can fire at ~half phase A
            with (
                tc.tile_pool(name="hw1s", bufs=3) as hs,
                tc.tile_pool(name="psumH1", bufs=2, space="PSUM") as ph,
            ):
                def hw1_tile(t):
                    ps = ph.tile([128, F1], f32, tag="psH1")
                    nc.tensor.matmul(ps[:, 0:128], x_g0[:, ts(t, 128)],
                                     Wsb["Wg1"][:], start=True, stop=True)
                    nc.tensor.matmul(ps[:, 128:256], x_f0[:, ts(t, 128)],
                                     Wsb["Wf1"][:], start=True, stop=True)
                    nc.tensor.matmul(ps[:, 256:384], x_ns0[:, ts(t, 128)],
                                     Wsb["Wns1"][:], start=True, stop=True)
                    hb = hs.tile([128, F1], bf16, tag="hw1b")
                    nc.scalar.activation(hb[:], ps[:], AF.Copy,
                                         scale=dinv[:, t:t + 1])
                    nc.sync.dma_start(hw1_loc[ts(t, 128), :], hb[:])

                t_done = 0
                for blk in range(NBLK):
                    for wname, bname, dest, m in [("Wg0", "bg0", x_g0, 128),
                                                  ("Wf0", "bf0", x_f0, 128)]:
                        ps = pa.tile([128, BLK], f32, tag=f"psA_{wname}")
                        nc.tensor.matmul(ps[:], Wsb[wname][:],
                                         dx_sb[:, ts(blk, BLK)],
                                         start=True, stop=True)
                        nc.scalar.activation(dest[:, ts(blk, BLK)], ps[:],
                                             AF.Lrelu, bias=Wsb[bname][:],
                                             alpha=0.01)
                    ps = pa.tile([64, BLK], f32, tag="psA_ns")
                    nc.tensor.matmul(ps[:], Wsb["Wns0"][:],
                                     cd_sb[:, ts(blk, BLK)],
                                     start=True, stop=True)
                    nc.scalar.activation(x_ns0[:, ts(blk, BLK)], ps[:],
                                         AF.Lrelu,
                                         bias=Wsb["bns0"][:], alpha=0.01)
                    t_cov = (blk + 1) * BLK // 128
                    for t in range(t_done, t_cov):
                        hw1_tile(t)
                    t_done = t_cov

        def split_ag(loc, full, F):
            rows = SH // AGSPLIT
            for k in range(AGSPLIT):
                nc.gpsimd.collective_compute(
                    "AllGather", OP.bypass,
                    replica_groups=[list(range(NCORES))],
                    ins=[loc[k * rows:(k + 1) * rows, :].opt()],
                    outs=[full[k * rows * NCORES:(k + 1) * rows * NCORES,
                               :].opt()])

        split_ag(hw1_loc, hw1_all, F1)
        if COPY_AG:
            hw1_use = nc.dram_tensor("hw1_copy", [NPAD, F1], bf16)
            nc.sync.dma_start(hw1_use[:, :], hw1_all[:, :])
        else:
            hw1_use = hw1_all

        def conv_round(hw_all, F, biasb, out_cb):
            """Per-tile gather + one-hot segment-sum + epilogue.
            out_cb(t, t2_ap) consumes the [128, F] pre-activation tile."""
            with (
                tc.tile_pool(name=f"r{F}", bufs=3) as rp,
                tc.tile_pool(name=f"rpsum{F}", bufs=2, space="PSUM") as rps,
            ):
                for t in range(NT):
                    C = int(CT[t]); cl = int(CL[t]); chh = int(CH[t])
                    base = int(ccol[t])
                    msg = rp.tile([128, C, F], bf16, tag="msg")
                    S = rp.tile([128, C, 128], bf16, tag="S")
                    nc.gpsimd.dma_gather(
                        out_ap=msg[:, 0:cl, :], in_ap=hw_all[:, :],
                        idxs_ap=idx[:, base * 8:(base + cl) * 8],
                        num_idxs=cl * 128, num_idxs_reg=cl * 128, elem_size=F,
                        single_packet=False, queue_num=t % NQUEUES)
                    nc.gpsimd.dma_gather(
                        out_ap=msg[:, cl:C, :], in_ap=hw_all[SPLIT:, :],
                        idxs_ap=idx[:, (base + cl) * 8:(base + C) * 8],
                        num_idxs=chh * 128, num_idxs_reg=chh * 128,
                        elem_size=F,
                        single_packet=False, queue_num=(t + 1) % NQUEUES)
                    if S_MODE == "bcast":
                        dsl = dstloc[:, base:base + C]
                        dst_b = AP(dstloc.tensor, dsl.offset,
                                   [dsl.ap[0], list(dsl.ap[1]), [0, 128]])
                        iota_b = AP(iota.tensor, iota[:].offset,
                                    [iota[:].ap[0], [0, C], list(iota[:].ap[1])])
                        nc.vector.tensor_tensor(S[:, :, :], dst_b, iota_b,
                                                OP.is_equal)
                    else:
                        for c in range(C):
                            nc.vector.tensor_scalar(
                                S[:, c, :], iota[:],
                                dstloc[:, base + c:base + c + 1], None,
                                OP.is_equal)
                    agg = rps.tile([128, F], f32, tag="agg")
                    for c in range(C):
                        nc.tensor.matmul(agg[:], S[:, c, :], msg[:, c, :],
                                         start=(c == 0), stop=(c == C - 1))
                    t2 = rp.tile([128, F], f32, tag="t2")
                    nc.vector.scalar_tensor_tensor(
                        t2[:], agg[:], dinv[:, t:t + 1], biasb[:],
                        OP.mult, OP.add)
                    out_cb(t, t2)

        # ---------------- Round 1: g1, f1, ns1 ------------------------------
        with (
            tc.tile_pool(name="keep1", bufs=1) as k1,
            tc.tile_pool(name="ep1", bufs=3) as e1,
            tc.tile_pool(name="ep1ps", bufs=2, space="PSUM") as e1ps,
        ):
            x_g1 = k1.tile([128, SH], f32)     # feature-major
            x_f1 = k1.tile([128, SH], f32)
            x_ns1 = k1.tile([128, SH], f32)    # node-major (per-tile blocks)

            with (
                tc.tile_pool(name="hw2s", bufs=3) as h2,
                tc.tile_pool(name="psumH2", bufs=2, space="PSUM") as ph2,
            ):
              def r1_out(t, t2):
                gf = e1.tile([128, 256], f32, tag="gf")
                nc.scalar.activation(gf[:], t2[:, 0:256], AF.Lrelu, alpha=0.01)
                nc.scalar.activation(x_ns1[:, ts(t, 128)], t2[:, 256:384],
                                     AF.Lrelu, alpha=0.01)
                for k, dest in ((0, x_g1), (1, x_f1)):
                    pt = e1ps.tile([128, 128], f32, tag="pt")
                    nc.tensor.transpose(pt[:], gf[:, ts(k, 128)], ident[:])
                    nc.scalar.copy(dest[:, ts(t, 128)], pt[:])
                # hW2 for this tile immediately, so split-AG2 can fire early
                ps = ph2.tile([128, F2], f32, tag="psH2")
                nc.tensor.matmul(ps[:, 0:128], x_g1[:, ts(t, 128)],
                                 Wsb["Wg2"][:], start=True, stop=True)
                nc.tensor.matmul(ps[:, 128:256], x_f1[:, ts(t, 128)],
                                 Wsb["Wf2"][:], start=True, stop=True)
                hb = h2.tile([128, F2], bf16, tag="hw2b")
                nc.scalar.activation(hb[:], ps[:], AF.Copy,
                                     scale=dinv[:, t:t + 1])
                nc.sync.dma_start(hw2_loc[ts(t, 128), :], hb[:])

              conv_round(hw1_use, F1, biasb1, r1_out)

            split_ag(hw2_loc, hw2_all, F2)
            if COPY_AG:
                hw2_use = nc.dram_tensor("hw2_copy", [NPAD, F2], bf16)
                nc.sync.dma_start(hw2_use[:, :], hw2_all[:, :])
            else:
                hw2_use = hw2_all

        # ---------------- Round 2: g2, f2; h_si -----------------------------
        with tc.tile_pool(name="keep2", bufs=1) as k2:
            x_g2 = k2.tile([128, SH], f32)     # feature-major
            h_si = k2.tile([128, SH], f32)     # feature-major
            h_ci = k2.tile([128, SH], f32)     # feature-major

            with (
                tc.tile_pool(name="ep2", bufs=3) as e2,
                tc.tile_pool(name="ep2ps", bufs=2, space="PSUM") as e2ps,
            ):
              r2_scope = True

                def r2_out(t, t2):
                gf2 = e2.tile([128, F2], f32, tag="gf2")
                nc.scalar.activation(gf2[:], t2[:], AF.Lrelu, alpha=0.01)
                pt = e2ps.tile([128, 128], f32, tag="pt2")
                nc.tensor.transpose(pt[:], gf2[:, 0:128], ident[:])
                nc.scalar.copy(x_g2[:, ts(t, 128)], pt[:])
                hsn = e2.tile([128, 128], f32, tag="hsn")
                nc.vector.tensor_tensor(hsn[:], gf2[:, 128:256],
                                        x_ns1[:, ts(t, 128)], OP.mult)
                pt2 = e2ps.tile([128, 128], f32, tag="pt3")
                nc.tensor.transpose(pt2[:], hsn[:], ident[:])
                nc.scalar.copy(h_si[:, ts(t, 128)], pt2[:])

              conv_round(hw2_all, F2, biasb2, r2_out)

            # ---------------- Phase C --------------------------------------
            with (
                tc.tile_pool(name="cstream", bufs=3) as cs,
                tc.tile_pool(name="cpsum", bufs=2, space="PSUM") as cps,
            ):
                dxC = cs.tile([64, SH], f32, tag="dxC", bufs=1)
                c1C = cs.tile([13, SH], f32, tag="c1C", bufs=1)
                c2C = cs.tile([13, SH], f32, tag="c2C", bufs=1)
                nc.sync.dma_start(dxC[:], dx_t[:, :])
                nc.sync.dma_start(c1C[:], cx1_t[:, :])
                nc.sync.dma_start(c2C[:], cx2_t[:, :])
                for blk in range(NBLK):
                    xs = []
                    for wname, bname, xin, k in [("Wd", "bd", dxC, 64),
                                                 ("Wc1", "bc1", c1C, 13),
                                                 ("Wc2", "bc2", c2C, 13)]:
                        ps = cps.tile([128, BLK], f32, tag=f"psC_{wname}")
                        nc.tensor.matmul(ps[:], Wsb[wname][:],
                                         xin[:, ts(blk, BLK)],
                                         start=True, stop=True)
                        xb = cs.tile([128, BLK], f32, tag=f"xb_{wname}")
                        nc.scalar.activation(xb[:], ps[:], AF.Lrelu,
                                             bias=Wsb[bname][:], alpha=0.01)
                        xs.append(xb)
                    psf = cps.tile([128, BLK], f32, tag="psfus")
                    rhss = [xs[0][:], xs[1][:], xs[2][:],
                            x_g2[:, ts(blk, BLK)]]
                    for b in range(4):
                        nc.tensor.matmul(psf[:], wfus[:, b, :], rhss[b],
                                         start=(b == 0), stop=(b == 3))
                    nc.scalar.activation(h_ci[:, ts(blk, BLK)], psf[:],
                                         AF.Lrelu, bias=Wsb["bfus"][:],
                                         alpha=0.01)

            with (
                tc.tile_pool(name="cfin", bufs=3) as cf,
                tc.tile_pool(name="cfps", bufs=1, space="PSUM") as fps,
            ):
                tci = cf.tile([64, SH], bf16, tag="tci", bufs=1)
                tsi = cf.tile([64, SH], bf16, tag="tsi", bufs=1)
                # stage 1: all Lrelu-stage matmuls (one act table)
                for blk in range(NBLK):
                    for (hsrc, w1, b1, dst) in [(h_ci, "Wl1", "bl1", tci),
                                                (h_si, "Wl3", "bl3", tsi)]:
                        ps = fps.tile([64, BLK], f32, tag="psl")
                        nc.tensor.matmul(ps[:], Wsb[w1][:],
                                         hsrc[:, ts(blk, BLK)],
                                         start=True, stop=True)
                        nc.scalar.activation(dst[:, ts(blk, BLK)], ps[:],
                                             AF.Lrelu, bias=Wsb[b1][:],
                                             alpha=0.01)
                # stage 2: sigmoids for s_ci/s_si rows
                sci = cf.tile([1, SH], bf16, tag="sci", bufs=1)
                ssi = cf.tile([1, SH], bf16, tag="ssi", bufs=1)
                for blk in range(NBLK):
                    for row, (src, w2, b2, dst) in enumerate(
                            [(tci, wl2bf, "bl2", sci),
                             (tsi, wl4bf, "bl4", ssi)]):
                        ps1 = fps.tile([1, BLK], f32, tag="psl2")
                        nc.tensor.matmul(ps1[:], w2[:],
                                         src[:, ts(blk, BLK)],
                                         start=True, stop=True)
                        srf = cf.tile([1, BLK], f32, tag="srf")
                        nc.scalar.activation(srf[:], ps1[:],
                                             AF.Sigmoid, bias=Wsb[b2][:])
                        nc.sync.dma_start(out_t[1 + row:2 + row, ts(blk, BLK)],
                                          srf[:])
                        nc.scalar.copy(dst[:, ts(blk, BLK)], srf[:])
                # stage 3: stack rows, Wr1 + Lrelu
                trf = cf.tile([2, SH], bf16, tag="trf", bufs=1)
                for blk in range(NBLK):
                    pss = fps.tile([2, BLK], f32, tag="pss")
                    nc.tensor.matmul(pss[:], selbf[:, 0:2],
                                     sci[:, ts(blk, BLK)],
                                     start=True, stop=False)
                    nc.tensor.matmul(pss[:], selbf[:, 2:4],
                                     ssi[:, ts(blk, BLK)],
                                     start=False, stop=True)
                    scat = cf.tile([2, BLK], f32, tag="scat")
                    nc.scalar.copy(scat[:], pss[:])
                    ps2 = fps.tile([2, BLK], f32, tag="psr")
                    nc.tensor.matmul(ps2[:], Wsb["Wr1"][:], scat[:],
                                     start=True, stop=True)
                    nc.scalar.activation(trf[:, ts(blk, BLK)], ps2[:],
                                         AF.Lrelu, bias=Wsb["br1"][:],
                                         alpha=0.01)
                # stage 4: Wr2 + sigmoid -> y (per-block DMA out)
                for blk in range(NBLK):
                    ps3 = fps.tile([1, BLK], f32, tag="psy")
                    nc.tensor.matmul(ps3[:], wr2bf[:],
                                     trf[:, ts(blk, BLK)],
                                     start=True, stop=True)
                    yb = cf.tile([1, BLK], f32, tag="yb")
                    nc.scalar.activation(yb[:], ps3[:],
                                         AF.Sigmoid, bias=Wsb["br2"][:])
                    nc.sync.dma_start(out_t[0:1, ts(blk, BLK)], yb[:])
            nc.sync.dma_start(out_t[3:131, :], h_ci[:])
            nc.sync.dma_start(out_t[131:259, :], h_si[:])

    nc.compile()
    return nc


# ----------------------------------------------------------------------------
# kernel entry
# ----------------------------------------------------------------------------

def kernel(**inputs):
    _setup_paths()
    import ml_dtypes
    from concourse import bass_utils
    bass_utils.upload_artifacts = lambda tmpdir: "local://" + tmpdir

    g = _prep_graph(np.asarray(inputs["edge_index"]))
    nc = _build(g)

    n2o = g["new2old"]
    real = n2o < N_REAL            # mask of real nodes in new order
    n2o_c = np.minimum(n2o, N_REAL - 1)

    def shard_fm(x, cols):
        """x [N_REAL, D] -> per-core feature-major [D, SH] f32 (pad rows 0)."""
        xp = np.asarray(x, np.float32)[n2o_c][:, cols]
        xp[~real] = 0.0
        return [np.ascontiguousarray(xp[c * SH:(c + 1) * SH].T)
                for c in range(NCORES)]

    dx = shard_fm(inputs["discrete_x"], slice(0, 64))
    cx1 = shard_fm(inputs["continous_x"], slice(0, 13))
    cx2 = shard_fm(inputs["continous_x"], slice(13, 26))
    cd = shard_fm(inputs["churn_date"], slice(0, 16))

    f = lambda a: np.ascontiguousarray(np.asarray(a, np.float32))
    col = lambda a: f(a).reshape(-1, 1)
    biasb1 = np.tile(np.concatenate([f(inputs["bg1"]), f(inputs["bf1"]),
                                     f(inputs["bns1"])])[None, :], (128, 1))
    biasb2 = np.tile(np.concatenate([f(inputs["bg2"]),
                                     f(inputs["bf2"])])[None, :], (128, 1))
    iota = np.tile(np.arange(128, dtype=np.float32).astype(
        ml_dtypes.bfloat16)[None, :], (128, 1))
    ident = np.eye(128, dtype=np.float32)

    common = dict(
        iota=iota, ident=ident, biasb1=biasb1, biasb2=biasb2,
        sel=np.array([[1.0, 0.0, 0.0, 1.0]], np.float32),
        selbf=np.array([[1.0, 0.0, 0.0, 1.0]], np.float32).astype(
            ml_dtypes.bfloat16),
        Wl2bf=f(inputs["Wl2"]).astype(ml_dtypes.bfloat16),
        Wl4bf=f(inputs["Wl4"]).astype(ml_dtypes.bfloat16),
        Wr2bf=f(inputs["Wr2"]).astype(ml_dtypes.bfloat16),
        Wd=f(inputs["Wd"]), Wc1=f(inputs["Wc1"]), Wc2=f(inputs["Wc2"]),
        Wg0=f(inputs["Wg0"]), Wf0=f(inputs["Wf0"]), Wns0=f(inputs["Wns0"]),
        Wg1=f(inputs["Wg1"]), Wf1=f(inputs["Wf1"]), Wns1=f(inputs["Wns1"]),
        Wg2=f(inputs["Wg2"]), Wf2=f(inputs["Wf2"]), Wfus=f(inputs["Wfus"]),
        Wl1=f(inputs["Wl1"]), Wl2=f(inputs["Wl2"]), Wl3=f(inputs["Wl3"]),
        Wl4=f(inputs["Wl4"]), Wr1=f(inputs["Wr1"]), Wr2=f(inputs["Wr2"]),
        bd=col(inputs["bd"]), bg0=col(inputs["bg0"]), bf0=col(inputs["bf0"]),
        bns0=col(inputs["bns0"]), bfus=col(inputs["bfus"]),
        bl1=col(inputs["bl1"]), bl2=col(inputs["bl2"]),
        bl3=col(inputs["bl3"]), bl4=col(inputs["bl4"]),
        br1=col(inputs["br1"]), br2=col(inputs["br2"]),
        bc1=col(inputs["bc1"]), bc2=col(inputs["bc2"]),
    )
    in_maps = []
    for c in range(NCORES):
        m = dict(common)
        m.update(dx_fm=dx[c], cx1_fm=cx1[c], cx2_fm=cx2[c], cd_fm=cd[c],
                 idx=g["IDX"][c], dstloc=g["DST"][c],
                 dinv_nm=np.ascontiguousarray(g["DINV"][c]))
        in_maps.append(m)

    try:
        res = bass_utils.run_bass_kernel_spmd(
            nc, in_maps, core_ids=list(range(NCORES)), trace=True)
    except Exception:
        res = bass_utils.run_bass_kernel_spmd(
            nc, in_maps, core_ids=list(range(NCORES)))
    kernel.last_exec_time_ns = res.exec_time_ns

    outs = [res.results[c]["out"] for c in range(NCORES)]
    full = np.concatenate(outs, axis=1)          # [259, NPAD]
    o2n = g["old2new"][:N_REAL]
    y = full[0, o2n].astype(np.float32)
    s_ci = full[1, o2n].astype(np.float32)[:, None]
    s_si = full[2, o2n].astype(np.float32)[:, None]
    h_ci = np.ascontiguousarray(full[3:131][:, o2n].T.astype(np.float32))
    h_si = np.ascontiguousarray(full[131:259][:, o2n].T.astype(np.float32))
    return (y, s_ci, s_si, h_ci, h_si)


# revision 38
# speedup vs baseline: 1.0493x; 1.0493x over previous
"""Trainium2 Bass kernel for the GCN message-passing model (8 NeuronCores).

Strategy (graph/data parallel per the sharding hint):
- Pad nodes 50000 -> 50176 = 8*49*128; permute node ids so every 128-node
  dst tile has a near-equal number of incoming edges (balanced one-hot
  matmul chunks), shard nodes 6272/core.
- Dense layers run feature-major (features on partitions) so bias+leaky_relu
  fuse into one ScalarEngine activation per block.
- Each GCN conv: per-core hW = (x @ W) * dinv[src] computed node-major,
  cast bf16, AllGather -> full [50176, F] gather table in every core's HBM.
  Self-loops are folded in as ordinary edges (out = lrelu(dinv*(agg) + b)
  with agg including the dinv*hW self term).
- Per dst tile: dma_gather the incoming edges' rows (int16 idx, split at
  row 32768), build one-hot S from uploaded dst-locals, segment-sum via
  PE matmuls accumulating in PSUM, epilogue on DVE+ACT.
- Convs are fused per round: round1 = {g1, f1, ns1} (384 gathered cols),
  round2 = {g2, f2} (256 cols) — one gather/S/matmul pass per round.
"""
import os
import os
import sys
import types

import numpy as np

N_REAL = 50000
NCORES = 8
NT = 49                      # dst tiles per core
SH = NT * 128                # 6272 nodes per core
NPAD = NCORES * SH           # 50176
SPLIT = 32768                # int16 gather index limit
F1, F2 = 384, 256            # gathered cols round1 / round2
S_MODE = "bcast"             # "bcast" | "chunk" one-hot build
NQUEUES = int(os.environ.get("K_NQUEUES", "2"))
COPY_AG = os.environ.get("K_COPY_AG", "0") == "1"
AGSPLIT = int(os.environ.get("K_AGSPLIT", "4"))
NQUEUES = int(os.environ.get("K_NQUEUES", "2"))
COPY_AG = os.environ.get("K_COPY_AG", "0") == "1"
AGSPLIT = int(os.environ.get("K_AGSPLIT", "4"))


def _setup_paths():
    if '/opt/trn_rl_repo' not in sys.path:
        sys.path.insert(0, '/opt/trn_rl_repo')
    if '/root/.axon_site' not in sys.path:
        sys.path.insert(0, '/root/.axon_site')
    if 'antenv.axon_hooks' not in sys.modules:
        box = [None]
        m = types.ModuleType('antenv.axon_hooks')
        m.set_axon_ntff_profile_hook = lambda h: box.__setitem__(0, h)
        m.get_axon_ntff_profile_hook = lambda: box[0]
        sys.modules['antenv.axon_hooks'] = m
        try:
            from trn_agent_boot.trn_boot import _ntff_profile_via_ctypes
            m.set_axon_ntff_profile_hook(
                _ntff_profile_via_ctypes('/opt/axon/libaxon_pjrt.so'))
        except Exception:
            pass


# ----------------------------------------------------------------------------
# Host-side graph preprocessing
# ----------------------------------------------------------------------------

def _balance_nodes(w_hi, w_lo):
    """Assign NPAD nodes to NT*NCORES tiles of 128, balancing the per-tile
    high-src edge count (tight: cap 6*128) primarily and total secondarily.
    Returns new2old [NPAD] (node order: tile t slots t*128..t*128+127)."""
    ntiles = NPAD // 128
    order = np.argsort(-(w_hi * 64 + w_lo), kind="stable")
    import heapq
    heap = [(0.0, t) for t in range(ntiles)]
    heapq.heapify(heap)
    counts = np.zeros(ntiles, np.int64)
    members = [[] for _ in range(ntiles)]
    for node in order:
        while True:
            s, t = heapq.heappop(heap)
            if counts[t] < 128:
                break
        members[t].append(node)
        counts[t] += 1
        if counts[t] < 128:
            heapq.heappush(heap, (s + float(w_hi[node]) + float(w_lo[node]) / 64.0, t))
    new2old = np.concatenate([np.array(m, np.int64) for m in members])
    return new2old


def _prep_graph(edge_index):
    src = np.asarray(edge_index[0], np.int64)
    dst = np.asarray(edge_index[1], np.int64)
    deg = np.bincount(dst, minlength=NPAD).astype(np.float32) + 1.0
    deg[N_REAL:] = 1.0
    dinv = (deg ** -0.5).astype(np.float32)

    # augmented edge list: real edges + one self loop per real node
    srcA = np.concatenate([src, np.arange(N_REAL)])
    dstA = np.concatenate([dst, np.arange(N_REAL)])

    # per-dst counts split by whether the src new-id will be >= SPLIT is
    # circular; approximate with old src ids (the permutation preserves the
    # low/high population ratio closely).
    hi_mask = srcA >= SPLIT
    w_hi = np.bincount(dstA[hi_mask], minlength=NPAD)
    w_lo = np.bincount(dstA[~hi_mask], minlength=NPAD)
    new2old = _balance_nodes(w_hi, w_lo)
    old2new = np.empty(NPAD, np.int64)
    old2new[new2old] = np.arange(NPAD)

    s_new = old2new[srcA]
    if AGSPLIT > 1:
        rows = SH // AGSPLIT
        r = s_new // SH
        i = s_new % SH
        s_new = (i // rows) * (NPAD // AGSPLIT) + r * rows + (i % rows)
    d_new = old2new[dstA]
    tile = d_new >> 7
    dloc = d_new & 127

    # group edges by (tile, low/high src)
    ntiles = NPAD // 128
    okey = tile * 2 + (s_new >= SPLIT)
    eorder = np.argsort(okey, kind="stable")
    s_new, dloc, okey = s_new[eorder], dloc[eorder], okey[eorder]
    n_low = np.bincount(tile[eorder] * 2 + (s_new >= SPLIT),
                        minlength=ntiles * 2).reshape(ntiles, 2)
    starts = np.zeros(ntiles * 2 + 1, np.int64)
    np.cumsum(n_low.reshape(-1), out=starts[1:])
    n_lo, n_hi = n_low[:, 0], n_low[:, 1]

    # per tile-slot chunk structure, maxed over cores (SPMD: one graph)
    n_lo_s = n_lo.reshape(NCORES, NT)
    n_hi_s = n_hi.reshape(NCORES, NT)
    CL = np.maximum(1, (n_lo_s.max(axis=0) + 127) // 128).astype(np.int64)
    CH = np.maximum(1, (n_hi_s.max(axis=0) + 127) // 128).astype(np.int64)
    CT = CL + CH                                     # chunks per tile slot
    totc = int(CT.sum())                             # chunk columns per core

    # per-core packed idx (int16, wrapped 16 + replicated x8) and dstloc (f32)
    IDX = np.zeros((NCORES, 128, totc * 8), np.int16)
    import ml_dtypes
    DST = np.full((NCORES, 128, totc), 255.0, ml_dtypes.bfloat16)
    ccol = np.zeros(NT + 1, np.int64)
    np.cumsum(CT, out=ccol[1:])
    for c in range(NCORES):
        for t in range(NT):
            g = c * NT + t
            base = ccol[t]
            for half, (cnt, coff, clen) in enumerate(
                    [(int(n_lo[g]), 0, int(CL[t])),
                     (int(n_hi[g]), int(CL[t]), int(CH[t]))]):
                e0 = starts[g * 2 + half]
                sl = s_new[e0:e0 + cnt] - (SPLIT if half else 0)
                dl = dloc[e0:e0 + cnt]
                nslot = clen * 128
                ii = np.arange(cnt)
                wrapped = np.zeros((16, clen * 8), np.int16)
                wrapped[ii % 16, ii // 16] = sl.astype(np.int16)
                IDX[c, :, (base + coff) * 8:(base + coff + clen) * 8] = \
                    np.tile(wrapped, (8, 1))
                dcol = np.full((128, clen), 255.0, ml_dtypes.bfloat16)
                dcol[ii % 128, ii // 128] = dl.astype(np.float32).astype(ml_dtypes.bfloat16)
                DST[c, :, base + coff:base + coff + clen] = dcol

    dinv_new = dinv[new2old]
    DINV = dinv_new.reshape(NCORES, NT, 128).transpose(0, 2, 1).copy()  # [C,128,NT]
    return dict(new2old=new2old, old2new=old2new, dinv_new=dinv_new,
                CL=CL, CH=CH, CT=CT, ccol=ccol, totc=totc,
                IDX=IDX, DST=DST, DINV=DINV)


# ----------------------------------------------------------------------------
# Bass graph
# ----------------------------------------------------------------------------

def _build(g):
    import concourse.bass as bass
    import concourse.bacc as bacc
    import concourse.tile as tile
    import concourse.mybir as mybir

    f32 = mybir.dt.float32
    bf16 = mybir.dt.bfloat16
    i16 = mybir.dt.int16
    AF = mybir.ActivationFunctionType
    OP = mybir.AluOpType
    AP = bass.AP
    ts = bass.ts

    CL, CH, CT, ccol, totc = g["CL"], g["CH"], g["CT"], g["ccol"], g["totc"]
    BLK = max(d for d in range(1, 513) if SH % d == 0)
    NBLK = SH // BLK

    nc = bacc.Bacc("TRN2", target_bir_lowering=False, debug=False,
                   num_devices=NCORES, num_swdge_queues=NQUEUES)

    def inp(name, shape, dt=f32):
        return nc.dram_tensor(name, shape, dt, kind="ExternalInput")

    dx_t = inp("dx_fm", [64, SH])
    cx1_t = inp("cx1_fm", [13, SH])
    cx2_t = inp("cx2_fm", [13, SH])
    cd_t = inp("cd_fm", [16, SH])
    idx_t = inp("idx", [128, totc * 8], i16)
    dstloc_t = inp("dstloc", [128, totc], bf16)
    dinv_t = inp("dinv_nm", [128, NT])
    iota_t = inp("iota", [128, 128], bf16)
    ident_t = inp("ident", [128, 128])
    biasb1_t = inp("biasb1", [128, F1])
    biasb2_t = inp("biasb2", [128, F2])
    sel_t = inp("sel", [1, 4])
    wl2bf_t = inp("Wl2bf", [64, 1], bf16)
    wl4bf_t = inp("Wl4bf", [64, 1], bf16)
    wr2bf_t = inp("Wr2bf", [2, 1], bf16)
    selbf_t = inp("selbf", [1, 4], bf16)
    W = {}
    for nm, sh in [("Wd", [64, 128]), ("Wc1", [13, 128]), ("Wc2", [13, 128]),
                   ("Wg0", [64, 128]), ("Wf0", [64, 128]), ("Wns0", [16, 64]),
                   ("Wg1", [128, 128]), ("Wf1", [128, 128]), ("Wns1", [64, 128]),
                   ("Wg2", [128, 128]), ("Wf2", [128, 128]),
                   ("Wfus", [512, 128]), ("Wl1", [128, 64]), ("Wl2", [64, 1]),
                   ("Wl3", [128, 64]), ("Wl4", [64, 1]), ("Wr1", [2, 2]),
                   ("Wr2", [2, 1]),
                   ("bd", [128, 1]), ("bg0", [128, 1]), ("bf0", [128, 1]),
                   ("bns0", [64, 1]), ("bfus", [128, 1]), ("bl1", [64, 1]),
                   ("bl2", [1, 1]), ("bl3", [64, 1]), ("bl4", [1, 1]),
                   ("br1", [2, 1]), ("br2", [1, 1]),
                   ("bc1", [128, 1]), ("bc2", [128, 1])]:
        W[nm] = inp(nm, sh)

    out_t = nc.dram_tensor("out", [259, SH], f32, kind="ExternalOutput")

    # internal DRAM
    hw1_loc = nc.dram_tensor("hw1_loc", [SH, F1], bf16)
    hw1_all = nc.dram_tensor("hw1_all", [NPAD, F1], bf16, addr_space="Shared")
    hw2_loc = nc.dram_tensor("hw2_loc", [SH, F2], bf16)
    hw2_all = nc.dram_tensor("hw2_all", [NPAD, F2], bf16, addr_space="Shared")

    with tile.TileContext(nc) as tc:
      with (
          tc.tile_pool(name="persist", bufs=1) as pp,
          tc.tile_pool(name="wpool", bufs=1) as wp,
      ):
        idx = pp.tile([128, totc * 8], i16)
        dstloc = pp.tile([128, totc], bf16)
        dinv = pp.tile([128, NT], f32)
        iota = pp.tile([128, 128], bf16)
        ident = pp.tile([128, 128], f32)
        biasb1 = pp.tile([128, F1], f32)
        biasb2 = pp.tile([128, F2], f32)
        sel = pp.tile([1, 4], f32)
        wl2bf = pp.tile([64, 1], bf16)
        wl4bf = pp.tile([64, 1], bf16)
        wr2bf = pp.tile([2, 1], bf16)
        selbf = pp.tile([1, 4], bf16)
        nc.sync.dma_start(selbf[:], selbf_t[:])
        nc.sync.dma_start(sel[:], sel_t[:])
        nc.sync.dma_start(wl2bf[:], wl2bf_t[:])
        nc.sync.dma_start(wl4bf[:], wl4bf_t[:])
        nc.sync.dma_start(wr2bf[:], wr2bf_t[:])
        nc.sync.dma_start(idx[:], idx_t[:])
        nc.sync.dma_start(dstloc[:], dstloc_t[:])
        nc.sync.dma_start(dinv[:], dinv_t[:])
        nc.sync.dma_start(iota[:], iota_t[:])
        nc.sync.dma_start(ident[:], ident_t[:])
        nc.sync.dma_start(biasb1[:], biasb1_t[:])
        nc.sync.dma_start(biasb2[:], biasb2_t[:])
        Wsb = {}
        for nm, t in W.items():
            if nm == "Wfus":
                continue
            sh = t.shape
            Wsb[nm] = wp.tile(list(sh), f32, name=f"W_{nm}")
            nc.sync.dma_start(Wsb[nm][:], t[:])
        wfus = wp.tile([128, 4, 128], f32)
        for b in range(4):
            nc.sync.dma_start(wfus[:, b, :], W["Wfus"][b * 128:(b + 1) * 128, :])

        # ---------------- Phase A: x_g0, x_f0, x_ns0 (feature-major) --------
        with (
            tc.tile_pool(name="keepA", bufs=1) as ka,
            tc.tile_pool(name="streamA", bufs=3) as sa,
            tc.tile_pool(name="psumA", bufs=2, space="PSUM") as pa,
        ):
            x_g0 = ka.tile([128, SH], f32)
            x_f0 = ka.tile([128, SH], f32)
            x_ns0 = ka.tile([64, SH], f32)
            dx_sb = sa.tile([64, SH], f32, tag="dx_sb", bufs=1)
            cd_sb = sa.tile([16, SH], f32, tag="cd_sb", bufs=1)
            nc.sync.dma_start(dx_sb[:], dx_t[:, :])
            nc.sync.dma_start(cd_sb[:], cd_t[:, :])
            # interleave hW1 emission so split-AG1 can fire at ~half phase A
            with (
                tc.tile_pool(name="hw1s", bufs=3) as hs,
                tc.tile_pool(name="psumH1", bufs=2, space="PSUM") as ph,
            ):
                def hw1_tile(t):
                    ps = ph.tile([128, F1], f32, tag="psH1")
                    nc.tensor.matmul(ps[:, 0:128], x_g0[:, ts(t, 128)],
                                     Wsb["Wg1"][:], start=True, stop=True)
                    nc.tensor.matmul(ps[:, 128:256], x_f0[:, ts(t, 128)],
                                     Wsb["Wf1"][:], start=True, stop=True)
                    nc.tensor.matmul(ps[:, 256:384], x_ns0[:, ts(t, 128)],
                                     Wsb["Wns1"][:], start=True, stop=True)
                    hb = hs.tile([128, F1], bf16, tag="hw1b")
                    nc.scalar.activation(hb[:], ps[:], AF.Copy,
                                         scale=dinv[:, t:t + 1])
                    nc.sync.dma_start(hw1_loc[ts(t, 128), :], hb[:])

                t_done = 0
                for blk in range(NBLK):
                    for wname, bname, dest, m in [("Wg0", "bg0", x_g0, 128),
                                                  ("Wf0", "bf0", x_f0, 128)]:
                        ps = pa.tile([128, BLK], f32, tag=f"psA_{wname}")
                        nc.tensor.matmul(ps[:], Wsb[wname][:],
                                         dx_sb[:, ts(blk, BLK)],
                                         start=True, stop=True)
                        nc.scalar.activation(dest[:, ts(blk, BLK)], ps[:],
                                             AF.Lrelu, bias=Wsb[bname][:],
                                             alpha=0.01)
                    ps = pa.tile([64, BLK], f32, tag="psA_ns")
                    nc.tensor.matmul(ps[:], Wsb["Wns0"][:],
                                     cd_sb[:, ts(blk, BLK)],
                                     start=True, stop=True)
                    nc.scalar.activation(x_ns0[:, ts(blk, BLK)], ps[:],
                                         AF.Lrelu,
                                         bias=Wsb["bns0"][:], alpha=0.01)
                    t_cov = (blk + 1) * BLK // 128
                    for t in range(t_done, t_cov):
                        hw1_tile(t)
                    t_done = t_cov

        def split_ag(loc, full, F):
            rows = SH // AGSPLIT
            for k in range(AGSPLIT):
                nc.gpsimd.collective_compute(
                    "AllGather", OP.bypass,
                    replica_groups=[list(range(NCORES))],
                    ins=[loc[k * rows:(k + 1) * rows, :].opt()],
                    outs=[full[k * rows * NCORES:(k + 1) * rows * NCORES,
                               :].opt()])

        split_ag(hw1_loc, hw1_all, F1)
        if COPY_AG:
            hw1_use = nc.dram_tensor("hw1_copy", [NPAD, F1], bf16)
            nc.sync.dma_start(hw1_use[:, :], hw1_all[:, :])
        else:
            hw1_use = hw1_all

        def conv_round(hw_all, F, biasb, out_cb):
            """Per-tile gather + one-hot segment-sum + epilogue.
            out_cb(t, t2_ap) consumes the [128, F] pre-activation tile."""
            with (
                tc.tile_pool(name=f"r{F}", bufs=3) as rp,
                tc.tile_pool(name=f"rpsum{F}", bufs=2, space="PSUM") as rps,
            ):
                for t in range(NT):
                    C = int(CT[t]); cl = int(CL[t]); chh = int(CH[t])
                    base = int(ccol[t])
                    msg = rp.tile([128, C, F], bf16, tag="msg")
                    S = rp.tile([128, C, 128], bf16, tag="S")
                    nc.gpsimd.dma_gather(
                        out_ap=msg[:, 0:cl, :], in_ap=hw_all[:, :],
                        idxs_ap=idx[:, base * 8:(base + cl) * 8],
                        num_idxs=cl * 128, num_idxs_reg=cl * 128, elem_size=F,
                        single_packet=False, queue_num=t % NQUEUES)
                    nc.gpsimd.dma_gather(
                        out_ap=msg[:, cl:C, :], in_ap=hw_all[SPLIT:, :],
                        idxs_ap=idx[:, (base + cl) * 8:(base + C) * 8],
                        num_idxs=chh * 128, num_idxs_reg=chh * 128,
                        elem_size=F,
                        single_packet=False, queue_num=(t + 1) % NQUEUES)
                    if S_MODE == "bcast":
                        dsl = dstloc[:, base:base + C]
                        dst_b = AP(dstloc.tensor, dsl.offset,
                                   [dsl.ap[0], list(dsl.ap[1]), [0, 128]])
                        iota_b = AP(iota.tensor, iota[:].offset,
                                    [iota[:].ap[0], [0, C], list(iota[:].ap[1])])
                        nc.vector.tensor_tensor(S[:, :, :], dst_b, iota_b,
                                                OP.is_equal)
                    else:
                        for c in range(C):
                            nc.vector.tensor_scalar(
                                S[:, c, :], iota[:],
                                dstloc[:, base + c:base + c + 1], None,
                                OP.is_equal)
                    agg = rps.tile([128, F], f32, tag="agg")
                    for c in range(C):
                        nc.tensor.matmul(agg[:], S[:, c, :], msg[:, c, :],
                                         start=(c == 0), stop=(c == C - 1))
                    t2 = rp.tile([128, F], f32, tag="t2")
                    nc.vector.scalar_tensor_tensor(
                        t2[:], agg[:], dinv[:, t:t + 1], biasb[:],
                        OP.mult, OP.add)
                    out_cb(t, t2)

        # ---------------- Round 1: g1, f1, ns1 ------------------------------
        with (
            tc.tile_pool(name="keep1", bufs=1) as k1,
            tc.tile_pool(name="ep1", bufs=3) as e1,
            tc.tile_pool(name="ep1ps", bufs=2, space="PSUM") as e1ps,
        ):
            x_g1 = k1.tile([128, SH], f32)     # feature-major
            x_f1 = k1.tile([128, SH], f32)
            x_ns1 = k1.tile([128, SH], f32)    # node-major (per-tile blocks)

            with (
                tc.tile_pool(name="hw2s", bufs=3) as h2,
                tc.tile_pool(name="psumH2", bufs=2, space="PSUM") as ph2,
            ):
              def r1_out(t, t2):
                gf = e1.tile([128, 256], f32, tag="gf")
                nc.scalar.activation(gf[:], t2[:, 0:256], AF.Lrelu, alpha=0.01)
                nc.scalar.activation(x_ns1[:, ts(t, 128)], t2[:, 256:384],
                                     AF.Lrelu, alpha=0.01)
                for k, dest in ((0, x_g1), (1, x_f1)):
                    pt = e1ps.tile([128, 128], f32, tag="pt")
                    nc.tensor.transpose(pt[:], gf[:, ts(k, 128)], ident[:])
                    nc.scalar.copy(dest[:, ts(t, 128)], pt[:])
                # hW2 for this tile immediately, so split-AG2 can fire early
                ps = ph2.tile([128, F2], f32, tag="psH2")
                nc.tensor.matmul(ps[:, 0:128], x_g1[:, ts(t, 128)],
                                 Wsb["Wg2"][:], start=True, stop=True)
                nc.tensor.matmul(ps[:, 128:256], x_f1[:, ts(t, 128)],
                                 Wsb["Wf2"][:], start=True, stop=True)
                hb = h2.tile([128, F2], bf16, tag="hw2b")
                nc.scalar.activation(hb[:], ps[:], AF.Copy,
                                     scale=dinv[:, t:t + 1])
                nc.sync.dma_start(hw2_loc[ts(t, 128), :], hb[:])

              conv_round(hw1_use, F1, biasb1, r1_out)

            split_ag(hw2_loc, hw2_all, F2)
            if COPY_AG:
                hw2_use = nc.dram_tensor("hw2_copy", [NPAD, F2], bf16)
                nc.sync.dma_start(hw2_use[:, :], hw2_all[:, :])
            else:
                hw2_use = hw2_all

        # ---------------- Round 2: g2, f2; h_si -----------------------------
        with tc.tile_pool(name="keep2", bufs=1) as k2:
            x_g2 = k2.tile([128, SH], f32)     # feature-major
            h_si = k2.tile([128, SH], f32)     # feature-major
            h_ci = k2.tile([128, SH], f32)     # feature-major

            with (
                tc.tile_pool(name="ep2", bufs=3) as e2,
                tc.tile_pool(name="ep2ps", bufs=2, space="PSUM") as e2ps,
            ):
              r2_scope = True

                def r2_out(t, t2):
                gf2 = e2.tile([128, F2], f32, tag="gf2")
                nc.scalar.activation(gf2[:], t2[:], AF.Lrelu, alpha=0.01)
                pt = e2ps.tile([128, 128], f32, tag="pt2")
                nc.tensor.transpose(pt[:], gf2[:, 0:128], ident[:])
                nc.scalar.copy(x_g2[:, ts(t, 128)], pt[:])
                hsn = e2.tile([128, 128], f32, tag="hsn")
                nc.vector.tensor_tensor(hsn[:], gf2[:, 128:256],
                                        x_ns1[:, ts(t, 128)], OP.mult)
                pt2 = e2ps.tile([128, 128], f32, tag="pt3")
                nc.tensor.transpose(pt2[:], hsn[:], ident[:])
                nc.scalar.copy(h_si[:, ts(t, 128)], pt2[:])

              conv_round(hw2_all, F2, biasb2, r2_out)

            # ---------------- Phase C --------------------------------------
            with (
                tc.tile_pool(name="cstream", bufs=3) as cs,
                tc.tile_pool(name="cpsum", bufs=2, space="PSUM") as cps,
            ):
                dxC = cs.tile([64, SH], f32, tag="dxC", bufs=1)
                c1C = cs.tile([13, SH], f32, tag="c1C", bufs=1)
                c2C = cs.tile([13, SH], f32, tag="c2C", bufs=1)
                nc.sync.dma_start(dxC[:], dx_t[:, :])
                nc.sync.dma_start(c1C[:], cx1_t[:, :])
                nc.sync.dma_start(c2C[:], cx2_t[:, :])
                for blk in range(NBLK):
                    xs = []
                    for wname, bname, xin, k in [("Wd", "bd", dxC, 64),
                                                 ("Wc1", "bc1", c1C, 13),
                                                 ("Wc2", "bc2", c2C, 13)]:
                        ps = cps.tile([128, BLK], f32, tag=f"psC_{wname}")
                        nc.tensor.matmul(ps[:], Wsb[wname][:],
                                         xin[:, ts(blk, BLK)],
                                         start=True, stop=True)
                        xb = cs.tile([128, BLK], f32, tag=f"xb_{wname}")
                        nc.scalar.activation(xb[:], ps[:], AF.Lrelu,
                                             bias=Wsb[bname][:], alpha=0.01)
                        xs.append(xb)
                    psf = cps.tile([128, BLK], f32, tag="psfus")
                    rhss = [xs[0][:], xs[1][:], xs[2][:],
                            x_g2[:, ts(blk, BLK)]]
                    for b in range(4):
                        nc.tensor.matmul(psf[:], wfus[:, b, :], rhss[b],
                                         start=(b == 0), stop=(b == 3))
                    nc.scalar.activation(h_ci[:, ts(blk, BLK)], psf[:],
                                         AF.Lrelu, bias=Wsb["bfus"][:],
                                         alpha=0.01)

            with (
                tc.tile_pool(name="cfin", bufs=3) as cf,
                tc.tile_pool(name="cfps", bufs=1, space="PSUM") as fps,
            ):
                tci = cf.tile([64, SH], bf16, tag="tci", bufs=1)
                tsi = cf.tile([64, SH], bf16, tag="tsi", bufs=1)
                # stage 1: all Lrelu-stage matmuls (one act table)
                for blk in range(NBLK):
                    for (hsrc, w1, b1, dst) in [(h_ci, "Wl1", "bl1", tci),
                                                (h_si, "Wl3", "bl3", tsi)]:
                        ps = fps.tile([64, BLK], f32, tag="psl")
                        nc.tensor.matmul(ps[:], Wsb[w1][:],
                                         hsrc[:, ts(blk, BLK)],
                                         start=True, stop=True)
                        nc.scalar.activation(dst[:, ts(blk, BLK)], ps[:],
                                             AF.Lrelu, bias=Wsb[b1][:],
                                             alpha=0.01)
                # stage 2: sigmoids for s_ci/s_si rows
                sci = cf.tile([1, SH], bf16, tag="sci", bufs=1)
                ssi = cf.tile([1, SH], bf16, tag="ssi", bufs=1)
                for blk in range(NBLK):
                    for row, (src, w2, b2, dst) in enumerate(
                            [(tci, wl2bf, "bl2", sci),
                             (tsi, wl4bf, "bl4", ssi)]):
                        ps1 = fps.tile([1, BLK], f32, tag="psl2")
                        nc.tensor.matmul(ps1[:], w2[:],
                                         src[:, ts(blk, BLK)],
                                         start=True, stop=True)
                        srf = cf.tile([1, BLK], f32, tag="srf")
                        nc.scalar.activation(srf[:], ps1[:],
                                             AF.Sigmoid, bias=Wsb[b2][:])
                        nc.sync.dma_start(out_t[1 + row:2 + row, ts(blk, BLK)],
                                          srf[:])
                        nc.scalar.copy(dst[:, ts(blk, BLK)], srf[:])
                # stage 3: stack rows, Wr1 + Lrelu
                trf = cf.tile([2, SH], bf16, tag="trf", bufs=1)
                for blk in range(NBLK):
                    pss = fps.tile([2, BLK], f32, tag="pss")
                    nc.tensor.matmul(pss[:], selbf[:, 0:2],
                                     sci[:, ts(blk, BLK)],
                                     start=True, stop=False)
                    nc.tensor.matmul(pss[:], selbf[:, 2:4],
                                     ssi[:, ts(blk, BLK)],
                                     start=False, stop=True)
                    scat = cf.tile([2, BLK], f32, tag="scat")
                    nc.scalar.copy(scat[:], pss[:])
                    ps2 = fps.tile([2, BLK], f32, tag="psr")
                    nc.tensor.matmul(ps2[:], Wsb["Wr1"][:], scat[:],
                                     start=True, stop=True)
                    nc.scalar.activation(trf[:, ts(blk, BLK)], ps2[:],
                                         AF.Lrelu, bias=Wsb["br1"][:],
                                         alpha=0.01)
                # stage 4: Wr2 + sigmoid -> y (per-block DMA out)
                for blk in range(NBLK):
                    ps3 = fps.tile([1, BLK], f32, tag="psy")
                    nc.tensor.matmul(ps3[:], wr2bf[:],
                                     trf[:, ts(blk, BLK)],
                                     start=True, stop=True)
                    yb = cf.tile([1, BLK], f32, tag="yb")
                    nc.scalar.activation(yb[:], ps3[:],
                                         AF.Sigmoid, bias=Wsb["br2"][:])
                    nc.sync.dma_start(out_t[0:1, ts(blk, BLK)], yb[:])
            nc.sync.dma_start(out_t[3:131, :], h_ci[:])
            nc.sync.dma_start(out_t[131:259, :], h_si[:])

    nc.compile()
    return nc


# ----------------------------------------------------------------------------
# kernel entry
# ----------------------------------------------------------------------------

def kernel(**inputs):
    _setup_paths()
    import ml_dtypes
    from concourse import bass_utils
    bass_utils.upload_artifacts = lambda tmpdir: "local://" + tmpdir

    g = _prep_graph(np.asarray(inputs["edge_index"]))
    nc = _build(g)

    n2o = g["new2old"]
    real = n2o < N_REAL            # mask of real nodes in new order
    n2o_c = np.minimum(n2o, N_REAL - 1)

    def shard_fm(x, cols):
        """x [N_REAL, D] -> per-core feature-major [D, SH] f32 (pad rows 0)."""
        xp = np.asarray(x, np.float32)[n2o_c][:, cols]
        xp[~real] = 0.0
        return [np.ascontiguousarray(xp[c * SH:(c + 1) * SH].T)
                for c in range(NCORES)]

    dx = shard_fm(inputs["discrete_x"], slice(0, 64))
    cx1 = shard_fm(inputs["continous_x"], slice(0, 13))
    cx2 = shard_fm(inputs["continous_x"], slice(13, 26))
    cd = shard_fm(inputs["churn_date"], slice(0, 16))

    f = lambda a: np.ascontiguousarray(np.asarray(a, np.float32))
    col = lambda a: f(a).reshape(-1, 1)
    biasb1 = np.tile(np.concatenate([f(inputs["bg1"]), f(inputs["bf1"]),
                                     f(inputs["bns1"])])[None, :], (128, 1))
    biasb2 = np.tile(np.concatenate([f(inputs["bg2"]),
                                     f(inputs["bf2"])])[None, :], (128, 1))
    iota = np.tile(np.arange(128, dtype=np.float32).astype(
        ml_dtypes.bfloat16)[None, :], (128, 1))
    ident = np.eye(128, dtype=np.float32)

    common = dict(
        iota=iota, ident=ident, biasb1=biasb1, biasb2=biasb2,
        sel=np.array([[1.0, 0.0, 0.0, 1.0]], np.float32),
        selbf=np.array([[1.0, 0.0, 0.0, 1.0]], np.float32).astype(
            ml_dtypes.bfloat16),
        Wl2bf=f(inputs["Wl2"]).astype(ml_dtypes.bfloat16),
        Wl4bf=f(inputs["Wl4"]).astype(ml_dtypes.bfloat16),
        Wr2bf=f(inputs["Wr2"]).astype(ml_dtypes.bfloat16),
        Wd=f(inputs["Wd"]), Wc1=f(inputs["Wc1"]), Wc2=f(inputs["Wc2"]),
        Wg0=f(inputs["Wg0"]), Wf0=f(inputs["Wf0"]), Wns0=f(inputs["Wns0"]),
        Wg1=f(inputs["Wg1"]), Wf1=f(inputs["Wf1"]), Wns1=f(inputs["Wns1"]),
        Wg2=f(inputs["Wg2"]), Wf2=f(inputs["Wf2"]), Wfus=f(inputs["Wfus"]),
        Wl1=f(inputs["Wl1"]), Wl2=f(inputs["Wl2"]), Wl3=f(inputs["Wl3"]),
        Wl4=f(inputs["Wl4"]), Wr1=f(inputs["Wr1"]), Wr2=f(inputs["Wr2"]),
        bd=col(inputs["bd"]), bg0=col(inputs["bg0"]), bf0=col(inputs["bf0"]),
        bns0=col(inputs["bns0"]), bfus=col(inputs["bfus"]),
        bl1=col(inputs["bl1"]), bl2=col(inputs["bl2"]),
        bl3=col(inputs["bl3"]), bl4=col(inputs["bl4"]),
        br1=col(inputs["br1"]), br2=col(inputs["br2"]),
        bc1=col(inputs["bc1"]), bc2=col(inputs["bc2"]),
    )
    in_maps = []
    for c in range(NCORES):
        m = dict(common)
        m.update(dx_fm=dx[c], cx1_fm=cx1[c], cx2_fm=cx2[c], cd_fm=cd[c],
                 idx=g["IDX"][c], dstloc=g["DST"][c],
                 dinv_nm=np.ascontiguousarray(g["DINV"][c]))
        in_maps.append(m)

    try:
        res = bass_utils.run_bass_kernel_spmd(
            nc, in_maps, core_ids=list(range(NCORES)), trace=True)
    except Exception:
        res = bass_utils.run_bass_kernel_spmd(
            nc, in_maps, core_ids=list(range(NCORES)))
    kernel.last_exec_time_ns = res.exec_time_ns

    outs = [res.results[c]["out"] for c in range(NCORES)]
    full = np.concatenate(outs, axis=1)          # [259, NPAD]
    o2n = g["old2new"][:N_REAL]
    y = full[0, o2n].astype(np.float32)
    s_ci = full[1, o2n].astype(np.float32)[:, None]
    s_si = full[2, o2n].astype(np.float32)[:, None]
    h_ci = np.ascontiguousarray(full[3:131][:, o2n].T.astype(np.float32))
    h_si = np.ascontiguousarray(full[131:259][:, o2n].T.astype(np.float32))
    return (y, s_ci, s_si, h_ci, h_si)
